# revision 39
# baseline (speedup 1.0000x reference)
"""Trainium2 Bass kernel for DeTrAttention (dense transformer MHA block).

Full op: out = softmax((q@Wq+bq)(k@Wk+bk)^T / sqrt(64)) (v@Wv+bv) @ Wo + bo
Shapes: q,k,v [B=2, S=2048, H=1024], NH=16 heads, HD=64.

Sharding (8 cores): data-parallel over batch (2 groups of 4 cores); within a
group each core owns a block of 512 query rows end-to-end (all heads), and
redundantly computes the K/V projections for its batch's full sequence.  No
collectives needed; every core writes a disjoint slice of the output.

On-chip layout strategy (avoids every on-chip transpose):
  - host passes q^T, k^T, v^T slices ([H, tokens], feature-major)
  - qp^T, kp^T computed with W stationary -> [head-major, tokens]
  - vp computed token-major via lhsT = v^T tiles
  - scores^T[kt, qt] = kp_h-slice.T @ qp_h  (contraction over head dim)
  - exp on scalar engine (no max subtraction: |scores| <= ~8 so exp is safe)
  - ctx^T[d, qt] accumulated over kt blocks with lhsT = vp (token-major);
    an appended ones-column in vp yields the softmax denominator Z in row 64
  - normalize via reciprocal + K=1 broadcast matmul + DVE multiply
  - out^T[o, t] = Wo-stationary matmul over ctx^T blocks; biases are applied
    with K=1 matmuls against a ones vector
Projections/scores run in float32r (fp32 data, ~16x better accuracy than
bf16, full PE rate for free dims >= 256); attention probabilities and V are
bf16 (end-to-end rel err ~2e-3).  DMAs are batched into multi-MB 3D transfers
(HWDGE descriptor generation costs ~625ns per dma_start regardless of size).
"""

import contextlib

import numpy as np

import concourse.bass as bass
import concourse.tile as tile
from concourse import bacc, mybir
from concourse.bass_utils import run_bass_kernel_spmd

F32 = mybir.dt.float32
F32R = mybir.dt.float32r
BF16 = mybir.dt.bfloat16

B, S, H, NH = 2, 2048, 1024, 16
HD = H // NH  # 64
N_CORES = 8
GROUPS = B  # batch groups
CPG = N_CORES // GROUPS  # cores per group (4)
SQ = S // CPG  # query rows per core (512)


def build_nc(s=S, h=H, nh=NH, sq=SQ, reps=0, upto=3, exp_sbuf=True, v2=False,
             ag=False, sreps=1):
    """Build the per-core Bass program (same program on all 8 cores).

    reps > 0 wraps the body in a hardware For_i loop (timing vehicle).
    sreps > 1 statically unrolls the body (timing vehicle that allows
    collectives, which cannot live inside control flow).
    """
    hd = h // nh
    assert hd == 64
    KB = h // 128          # contraction blocks over h_in
    KTB = s // 128         # key-token 128-blocks
    WN = min(512, s)       # free-dim block for kpT/vp projections
    NKT = s // WN          # kt 512-blocks
    TPW = WN // 128        # token 128-blocks per 512-block
    MB = h // 128          # ho 128-blocks total
    MH = MB // 2           # ho blocks per half
    HPH = nh // 2          # heads per half
    WH = h // 2            # ho columns per half
    assert WH <= 512 and sq <= 512

    nc = bacc.Bacc("TRN2", target_bir_lowering=False, debug=False,
                   num_devices=8 if ag else 1)

    SL = s // 4 if ag else s   # local K/V token span (4-rank AllGather)
    qT = nc.dram_tensor("qT", [h, sq], F32R, kind="ExternalInput").ap()
    kT = nc.dram_tensor("kT", [h, SL], F32R, kind="ExternalInput").ap()
    vT = nc.dram_tensor("vT", [h, SL], F32R, kind="ExternalInput").ap()
    Wq = nc.dram_tensor("Wq", [h, h], F32R, kind="ExternalInput").ap()
    Wk = nc.dram_tensor("Wk", [h, h], F32R, kind="ExternalInput").ap()
    Wv = nc.dram_tensor("Wv", [h, h], F32R, kind="ExternalInput").ap()
    Wo = nc.dram_tensor("Wo", [h, h], F32R, kind="ExternalInput").ap()
    bq = nc.dram_tensor("bq", [1, h], F32R, kind="ExternalInput").ap()
    bk = nc.dram_tensor("bk", [1, h], F32R, kind="ExternalInput").ap()
    bv = nc.dram_tensor("bv", [1, h], F32R, kind="ExternalInput").ap()
    bo = nc.dram_tensor("bo", [1, h], F32R, kind="ExternalInput").ap()
    outT = nc.dram_tensor("outT", [h, sq], F32, kind="ExternalOutput").ap()

    # [p, kb, cols] views (partition-major) so whole weights load in one DMA
    qT_p = qT.rearrange("(kb p) t -> p kb t", p=128)
    kT_p = kT.rearrange("(kb p) t -> p kb t", p=128)
    vT_p = vT.rearrange("(kb p) t -> p kb t", p=128)
    Wq_p = Wq.rearrange("(kb p) o -> p kb o", p=128)
    Wk_p = Wk.rearrange("(kb p) o -> p kb o", p=128)
    Wv_p = Wv.rearrange("(kb p) o -> p kb o", p=128)
    Wo_p = Wo.rearrange("(kb p) o -> p kb o", p=128)
    outT_p = outT.rearrange("(ob p) t -> p ob t", p=128)

    with tile.TileContext(nc) as tc:
        with tc.tile_pool(name="persist", bufs=1) as persist, \
             tc.tile_pool(name="consts", bufs=1) as consts, \
             tc.tile_pool(name="stream", bufs=2) as stream, \
             tc.tile_pool(name="wqo", bufs=2) as wqo, \
             tc.tile_pool(name="exps", bufs=8) as exps, \
             tc.tile_pool(name="zrp", bufs=2) as zrp, \
             tc.tile_pool(name="dramp", bufs=1, space="DRAM") as dramp, \
             tc.tile_pool(name="ps512", bufs=2, space="PSUM") as ps512, \
             tc.tile_pool(name="ps1024", bufs=2, space="PSUM") as ps1024, \
             tc.tile_pool(name="psc", bufs=2, space="PSUM") as pscp:

            ones_f32 = consts.tile([1, max(sq, WN, 128)], F32)
            nc.vector.memset(ones_f32, 1.0)
            ones = consts.tile([1, max(sq, WN, 128)], F32R)
            nc.vector.tensor_copy(ones, ones_f32)
            ones_rep = consts.tile([128, HPH], F32)
            nc.vector.memset(ones_rep, 1.0)
            bq_sb = consts.tile([1, h], F32R, tag="bq")
            bk_sb = consts.tile([1, h], F32R, tag="bk")
            bv_sb = consts.tile([1, h], F32R, tag="bv")
            bo_sb = consts.tile([1, h], F32R, tag="bo")
            nc.sync.dma_start(out=bq_sb, in_=bq)
            nc.sync.dma_start(out=bk_sb, in_=bk)
            nc.sync.dma_start(out=bv_sb, in_=bv)
            nc.sync.dma_start(out=bo_sb, in_=bo)

            # normalized merged-head context, transposed: [128, MB, sq]
            ctxnT = persist.tile([128, MB, sq], F32R, tag="ctxnT")

            loop_cm = tc.For_i(0, reps, 1) if reps else contextlib.nullcontext()
            with loop_cm:
              for _srep in range(sreps):
                for hf in range(2):
                    hoff = hf * WH

                    # ---- resident weights for this half (1 DMA each;
                    # Wv deferred past the first kT tile so the first kpT
                    # matmul isn't queued behind 4MB of weight DMA) ----
                    Wk_sb = persist.tile([128, KB, WH], F32R, tag="wk")
                    Wv_sb = persist.tile([128, KB, WH], F32R, tag="wv")
                    nc.sync.dma_start(out=Wk_sb,
                                      in_=Wk_p[:, :, hoff:hoff + WH])

                    # ---- kp^T projection: [ho(128 x MH), kt] ----
                    kpT = persist.tile([128, MH, s], F32R, tag="kpT")
                    if ag:
                        agki = dramp.tile([128, MH, SL], F32R, tag="agki")
                        agko = dramp.tile([4, 128, MH, SL], F32R, tag="agko")
                    for n in range(SL // WN):
                        kt_t = stream.tile([128, KB, WN], F32R, tag="st2mb",
                                           name="kt_t")
                        nc.sync.dma_start(
                            out=kt_t, in_=kT_p[:, :, n * WN:(n + 1) * WN])
                        for m in range(MH):
                            ps = ps512.tile([128, WN], F32, tag="ps512")
                            for kb in range(KB):
                                nc.tensor.matmul(
                                    ps, Wk_sb[:, kb, m * 128:(m + 1) * 128],
                                    kt_t[:, kb, :], start=(kb == 0),
                                    stop=False)
                            nc.tensor.matmul(
                                ps,
                                bk_sb[0:1, hoff + m * 128:hoff + (m + 1) * 128],
                                ones[0:1, 0:WN], start=False, stop=True)
                            if ag:
                                kst = wqo.tile([128, WN], F32R, tag="kst")
                                nc.vector.tensor_copy(kst, ps)
                                nc.sync.dma_start(
                                    out=agki[:, m, n * WN:(n + 1) * WN],
                                    in_=kst)
                            else:
                                nc.vector.tensor_copy(
                                    kpT[:, m, n * WN:(n + 1) * WN], ps)

                    # ---- vp projection (token-major) + ones column ----
                    # vp[kt-part, ktb, head_local, 0:64] ; [.., 64] = 1.0
                    nc.sync.dma_start(out=Wv_sb,
                                      in_=Wv_p[:, :, hoff:hoff + WH])
                    vp = persist.tile([128, KTB, HPH, hd + 1], BF16, tag="vp")
                    if ag:
                        KTL = SL // 128
                        vps = persist.tile([128, KTL, HPH, hd + 1], BF16,
                                           tag="vps")
                        agvi = dramp.tile([128, KTL, HPH, hd + 1], BF16,
                                          tag="agvi")
                        agvo = dramp.tile([4, 128, KTL, HPH, hd + 1], BF16,
                                          tag="agvo")
                    else:
                        vps = vp
                    for n in range(SL // WN):
                        vt_t = stream.tile([128, KB, WN], F32R, tag="st2mb",
                                           name="vt_t")
                        nc.sync.dma_start(
                            out=vt_t, in_=vT_p[:, :, n * WN:(n + 1) * WN])
                        for st in range(TPW):
                            t = n * TPW + st
                            nc.vector.tensor_copy(vps[:, t, :, hd:hd + 1],
                                                  ones_rep)
                            ps = ps512.tile([128, WH], F32, tag="ps512")
                            for kb in range(KB):
                                nc.tensor.matmul(
                                    ps, vt_t[:, kb, st * 128:(st + 1) * 128],
                                    Wv_sb[:, kb, :], start=(kb == 0),
                                    stop=False)
                            nc.tensor.matmul(ps, ones[0:1, 0:128],
                                             bv_sb[0:1, hoff:hoff + WH],
                                             start=False, stop=True)
                            nc.vector.tensor_copy(
                                vps[:, t, :, 0:hd],
                                ps.rearrange("p (hh d) -> p hh d", d=hd))
                    if ag:
                        nc.sync.dma_start(out=agvi, in_=vps)
                        nc.gpsimd.collective_compute(
                            "AllGather", mybir.AluOpType.bypass,
                            ins=[agki.opt()], outs=[agko.opt()],
                            replica_groups=[[0, 1, 2, 3], [4, 5, 6, 7]])
                        nc.gpsimd.collective_compute(
                            "AllGather", mybir.AluOpType.bypass,
                            ins=[agvi.opt()], outs=[agvo.opt()],
                            replica_groups=[[0, 1, 2, 3], [4, 5, 6, 7]])

                    # ---- qp^T projection: [ho(128 x MH), qt] ----
                    qT_sb = stream.tile([128, KB, sq], F32R, tag="st2mb",
                                        name="qT_sb")
                    nc.sync.dma_start(out=qT_sb, in_=qT_p)
                    qpT = persist.tile([128, MH, sq], F32R, tag="qpT")
                    wq_t = stream.tile([128, KB, WH], F32R, tag="st2mb",
                                       name="wq_t")
                    nc.sync.dma_start(out=wq_t,
                                      in_=Wq_p[:, :, hoff:hoff + WH])
                    for m in range(MH):
                        ps = ps512.tile([128, sq], F32, tag="ps512")
                        for kb in range(KB):
                            nc.tensor.matmul(
                                ps, wq_t[:, kb, m * 128:(m + 1) * 128],
                                qT_sb[:, kb, :], start=(kb == 0), stop=False)
                        nc.tensor.matmul(
                            ps,
                            bq_sb[0:1, hoff + m * 128:hoff + (m + 1) * 128],
                            ones[0:1, 0:sq], start=False, stop=True)
                        nc.vector.tensor_copy(qpT[:, m, :], ps)

                    if ag:
                        nc.sync.dma_start(
                            out=kpT.rearrange("p m (r t) -> p m r t", r=4),
                            in_=agko.rearrange("r p m t -> p m r t"))
                        nc.sync.dma_start(
                            out=vp.rearrange("p (r k) h c -> p r k h c", r=4),
                            in_=agvo.rearrange("r p k h c -> p r k h c"))

                    if upto < 2:
                        # consume proj outputs so DCE keeps them (timing mode)
                        nc.sync.dma_start(out=outT_p[:, 0, :],
                                          in_=kpT[:, 0, 0:sq].bitcast(F32))
                        nc.sync.dma_start(out=outT_p[:, 1, :],
                                          in_=qpT[:, 0, :].bitcast(F32))
                        nc.gpsimd.dma_start(
                            out=outT_p[:, 2, 0:65],
                            in_=vp[:, hf, 0, :])
                        continue
                    # ---- attention, processed in head pairs ----
                    # scores 2 ktb per 2-bank psum tile -> one exp per
                    # [128, 2*sq]; ctx MMs software-pipelined one chunk
                    # behind so ACT streams without PE ping-pong.
                    CH = min(4 if v2 else 2, KTB)  # ktb per chunk
                    n_ps_bufs = 1 if v2 else 2
                    for pr in range(HPH // 2):
                        m = pr  # ho block of this pair within the half
                        psc = [pscp.tile([hd + 1, sq], F32, tag="psc",
                                         name=f"psc{j}") for j in range(2)]
                        prev = None
                        for cc in range(KTB // CH):
                            p1s = [ps1024.tile([128, CH, sq], F32,
                                               tag="ps1024", bufs=n_ps_bufs,
                                               name=f"p1_{j}")
                                   for j in range(2)]
                            # interleave the two heads' K=64 matmuls so
                            # consecutive MMs hit disjoint PE row groups
                            for i in range(CH):
                                ktb = cc * CH + i
                                for j, roff in enumerate((0, 64)):
                                    nc.tensor.matmul(
                                        p1s[j][:, i, :],
                                        kpT[roff:roff + 64, m,
                                            ktb * 128:(ktb + 1) * 128],
                                        qpT[roff:roff + 64, m, :],
                                        start=True, stop=True)
                            ets = []
                            for j in range(2):
                                # load-balance the exp feed: even chunks go
                                # through a DVE psum->sbuf staging copy, odd
                                # chunks exp straight from PSUM on ACT, so
                                # neither engine carries the whole stream
                                if exp_sbuf and (cc + j) % 2 == 0:
                                    sc = exps.tile([128, CH, sq], F32,
                                                   tag="sc_t", bufs=2,
                                                   name=f"sc_{j}")
                                    nc.vector.tensor_copy(sc, p1s[j])
                                else:
                                    sc = p1s[j]
                                et = exps.tile([128, CH, sq], BF16,
                                               tag="exp_t",
                                               bufs=4 if v2 else 8,
                                               name=f"et_{j}")
                                nc.scalar.activation(
                                    out=et, in_=sc,
                                    func=mybir.ActivationFunctionType.Exp)
                                ets.append(et)
                            if prev is not None:
                                pcc, pets = prev
                                for j in range(2):
                                    for i in range(CH):
                                        ktb = pcc * CH + i
                                        nc.tensor.matmul(
                                            psc[j],
                                            vp[:, ktb, 2 * pr + j, :],
                                            pets[j][:, i, :],
                                            start=(ktb == 0),
                                            stop=(ktb == KTB - 1))
                            prev = (cc, ets)
                        pcc, pets = prev
                        for j in range(2):
                            for i in range(CH):
                                ktb = pcc * CH + i
                                nc.tensor.matmul(
                                    psc[j], vp[:, ktb, 2 * pr + j, :],
                                    pets[j][:, i, :],
                                    start=(ktb == 0),
                                    stop=(ktb == KTB - 1))
                        # normalize: ctxn = ctx * (1/Z) bcast over partitions
                        # (Z-broadcast on the otherwise-idle gpsimd engine:
                        # frees a PE matmul + a DVE copy per head and lets
                        # the ctx PSUM banks release sooner)
                        for j, roff in enumerate((0, 64)):
                            zr = zrp.tile([1, sq], F32, tag="zr", bufs=2)
                            with nc.allow_low_precision(
                                    reason="1/Z of softmax; feeds a DVE mul"):
                                nc.vector.reciprocal(zr, psc[j][hd:hd + 1, :])
                            zb = zrp.tile([64, sq], F32, tag="zb", bufs=2)
                            nc.gpsimd.partition_broadcast(zb, zr)
                            nc.vector.tensor_mul(
                                ctxnT[roff:roff + 64, hf * MH + m, :],
                                psc[j][0:hd, :], zb)

                if upto < 3:
                    if upto == 2:
                        nc.sync.dma_start(out=outT_p[:, 0, :],
                                          in_=ctxnT[:, 0, :].bitcast(F32))
                    continue_marker = True
                # ---- output projection: outT[o, t] ----
                for ob in range(MB if upto >= 3 else 0):
                    wo_t = wqo.tile([128, KB, 128], F32R, tag="wo_t")
                    nc.sync.dma_start(
                        out=wo_t, in_=Wo_p[:, :, ob * 128:(ob + 1) * 128])
                    po = ps512.tile([128, sq], F32, tag="ps512")
                    for mb in range(MB):
                        nc.tensor.matmul(po, wo_t[:, mb, :], ctxnT[:, mb, :],
                                         start=(mb == 0), stop=False)
                    nc.tensor.matmul(po,
                                     bo_sb[0:1, ob * 128:(ob + 1) * 128],
                                     ones[0:1, 0:sq], start=False, stop=True)
                    ot = wqo.tile([128, sq], F32, tag="ot")
                    nc.vector.tensor_copy(ot, po)
                    nc.sync.dma_start(out=outT_p[:, ob, :], in_=ot)

    nc.compile()
    return nc


def shard_inputs(q, k, v, Wq, bq, Wk, bk, Wv, bv, Wo, bo,
                 s=S, h=H, sq=SQ, n_cores=N_CORES, cpg=CPG, ag=False):
    """Host-side sharding: per-core input dicts (numpy, fp32, contiguous)."""
    scale = np.float32(1.0 / np.sqrt(HD))
    c32 = lambda a: np.ascontiguousarray(a, dtype=np.float32)
    Wq_s, bq_s = c32(Wq) * scale, c32(bq).reshape(1, h) * scale
    Wk_c, bk_c = c32(Wk), c32(bk).reshape(1, h)
    Wv_c, bv_c = c32(Wv), c32(bv).reshape(1, h)
    Wo_c, bo_c = c32(Wo), c32(bo).reshape(1, h)
    kT_b = [c32(k[b].T) for b in range(q.shape[0])]
    vT_b = [c32(v[b].T) for b in range(q.shape[0])]
    in_maps = []
    for c in range(n_cores):
        b, r0 = c // cpg, (c % cpg) * sq
        rr = c % cpg
        ksl = kT_b[b] if not ag else c32(kT_b[b][:, rr * (s // 4):(rr + 1) * (s // 4)])
        vsl = vT_b[b] if not ag else c32(vT_b[b][:, rr * (s // 4):(rr + 1) * (s // 4)])
        in_maps.append({
            "qT": c32(q[b, r0:r0 + sq, :].T),
            "kT": ksl, "vT": vsl,
            "Wq": Wq_s, "bq": bq_s, "Wk": Wk_c, "bk": bk_c,
            "Wv": Wv_c, "bv": bv_c, "Wo": Wo_c, "bo": bo_c,
        })
    return in_maps


_NC_CACHE = {}


AG = False  # set True to use the 4-rank AllGather variant


def get_nc():
    if "nc" not in _NC_CACHE:
        _NC_CACHE["nc"] = build_nc(ag=AG)
    return _NC_CACHE["nc"]


def kernel(q, k, v, Wq, bq, Wk, bk, Wv, bv, Wo, bo):
    q, k, v = np.asarray(q), np.asarray(k), np.asarray(v)
    in_maps = shard_inputs(q, k, v, Wq, bq, Wk, bk, Wv, bv, Wo, bo, ag=AG)
    nc = get_nc()
    res = run_bass_kernel_spmd(nc, in_maps, core_ids=list(range(N_CORES)))
    out = np.empty((B, S, H), dtype=np.float32)
    for c in range(N_CORES):
        b, r0 = c // CPG, (c % CPG) * SQ
        out[b, r0:r0 + SQ, :] = res.results[c]["outT"].T
    return out


# revision 42
# speedup vs baseline: 1.3786x; 1.3786x over previous
"""Trainium2 Bass kernel for DeTrAttention (dense transformer MHA block).

Full op: out = softmax((q@Wq+bq)(k@Wk+bk)^T / sqrt(64)) (v@Wv+bv) @ Wo + bo
Shapes: q,k,v [B=2, S=2048, H=1024], NH=16 heads, HD=64.

Sharding (8 cores): data-parallel over batch (2 groups of 4 cores); within a
group each core owns a block of 512 query rows end-to-end (all heads), and
redundantly computes the K/V projections for its batch's full sequence.  No
collectives needed; every core writes a disjoint slice of the output.

On-chip layout strategy (avoids every on-chip transpose):
  - host passes q^T, k^T, v^T slices ([H, tokens], feature-major)
  - qp^T, kp^T computed with W stationary -> [head-major, tokens]
  - vp computed token-major via lhsT = v^T tiles
  - scores^T[kt, qt] = kp_h-slice.T @ qp_h  (contraction over head dim)
  - exp on scalar engine (no max subtraction: |scores| <= ~8 so exp is safe)
  - ctx^T[d, qt] accumulated over kt blocks with lhsT = vp (token-major);
    an appended ones-column in vp yields the softmax denominator Z in row 64
  - normalize via reciprocal + K=1 broadcast matmul + DVE multiply
  - out^T[o, t] = Wo-stationary matmul over ctx^T blocks; biases are applied
    with K=1 matmuls against a ones vector
Projections/scores run in float32r (fp32 data, ~16x better accuracy than
bf16, full PE rate for free dims >= 256); attention probabilities and V are
bf16 (end-to-end rel err ~2e-3).  DMAs are batched into multi-MB 3D transfers
(HWDGE descriptor generation costs ~625ns per dma_start regardless of size).
"""

import contextlib

import numpy as np

import concourse.bass as bass
import concourse.tile as tile
from concourse import bacc, mybir
from concourse.bass_utils import run_bass_kernel_spmd

F32 = mybir.dt.float32
F32R = mybir.dt.float32r
BF16 = mybir.dt.bfloat16

B, S, H, NH = 2, 2048, 1024, 16
HD = H // NH  # 64
N_CORES = 8
GROUPS = B  # batch groups
CPG = N_CORES // GROUPS  # cores per group (4)
SQ = S // CPG  # query rows per core (512)


def build_nc(s=S, h=H, nh=NH, sq=SQ, reps=0, upto=3, exp_sbuf=True, v2=False,
             ag=False, sreps=1):
    """Build the per-core Bass program (same program on all 8 cores).

    reps > 0 wraps the body in a hardware For_i loop (timing vehicle).
    sreps > 1 statically unrolls the body (timing vehicle that allows
    collectives, which cannot live inside control flow).
    """
    hd = h // nh
    assert hd == 64
    KB = h // 128          # contraction blocks over h_in
    KTB = s // 128         # key-token 128-blocks
    WN = min(512, s)       # free-dim block for kpT/vp projections
    NKT = s // WN          # kt 512-blocks
    TPW = WN // 128        # token 128-blocks per 512-block
    MB = h // 128          # ho 128-blocks total
    MH = MB // 2           # ho blocks per half
    HPH = nh // 2          # heads per half
    WH = h // 2            # ho columns per half
    assert WH <= 512 and sq <= 512

    nc = bacc.Bacc("TRN2", target_bir_lowering=False, debug=False,
                   num_devices=8 if ag else 1)

    SL = s // 4 if ag else s   # local K/V token span (4-rank AllGather)
    qT = nc.dram_tensor("qT", [h, sq], F32R, kind="ExternalInput").ap()
    kT = nc.dram_tensor("kT", [h, SL], F32R, kind="ExternalInput").ap()
    vT = nc.dram_tensor("vT", [h, SL], F32R, kind="ExternalInput").ap()
    Wq = nc.dram_tensor("Wq", [h, h], F32R, kind="ExternalInput").ap()
    Wk = nc.dram_tensor("Wk", [h, h], F32R, kind="ExternalInput").ap()
    Wv = nc.dram_tensor("Wv", [h, h], F32R, kind="ExternalInput").ap()
    Wo = nc.dram_tensor("Wo", [h, h], F32R, kind="ExternalInput").ap()
    bq = nc.dram_tensor("bq", [1, h], F32R, kind="ExternalInput").ap()
    bk = nc.dram_tensor("bk", [1, h], F32R, kind="ExternalInput").ap()
    bv = nc.dram_tensor("bv", [1, h], F32R, kind="ExternalInput").ap()
    bo = nc.dram_tensor("bo", [1, h], F32R, kind="ExternalInput").ap()
    outT = nc.dram_tensor("outT", [h, sq], F32, kind="ExternalOutput").ap()

    # [p, kb, cols] views (partition-major) so whole weights load in one DMA
    qT_p = qT.rearrange("(kb p) t -> p kb t", p=128)
    kT_p = kT.rearrange("(kb p) t -> p kb t", p=128)
    vT_p = vT.rearrange("(kb p) t -> p kb t", p=128)
    Wq_p = Wq.rearrange("(kb p) o -> p kb o", p=128)
    Wk_p = Wk.rearrange("(kb p) o -> p kb o", p=128)
    Wv_p = Wv.rearrange("(kb p) o -> p kb o", p=128)
    Wo_p = Wo.rearrange("(kb p) o -> p kb o", p=128)
    outT_p = outT.rearrange("(ob p) t -> p ob t", p=128)

    with tile.TileContext(nc) as tc:
        with tc.tile_pool(name="persist", bufs=1) as persist, \
             tc.tile_pool(name="consts", bufs=1) as consts, \
             tc.tile_pool(name="stream", bufs=2) as stream, \
             tc.tile_pool(name="wqo", bufs=2) as wqo, \
             tc.tile_pool(name="exps", bufs=8) as exps, \
             tc.tile_pool(name="zrp", bufs=2) as zrp, \
             tc.tile_pool(name="dramp", bufs=1, space="DRAM") as dramp, \
             tc.tile_pool(name="ps512", bufs=2, space="PSUM") as ps512, \
             tc.tile_pool(name="ps1024", bufs=2, space="PSUM") as ps1024, \
             tc.tile_pool(name="psc", bufs=2, space="PSUM") as pscp:

            ones_f32 = consts.tile([1, max(sq, WN, 128)], F32)
            nc.vector.memset(ones_f32, 1.0)
            ones = consts.tile([1, max(sq, WN, 128)], F32R)
            nc.vector.tensor_copy(ones, ones_f32)
            ones_rep = consts.tile([128, HPH], F32)
            nc.vector.memset(ones_rep, 1.0)
            bq_sb = consts.tile([1, h], F32R, tag="bq")
            bk_sb = consts.tile([1, h], F32R, tag="bk")
            bv_sb = consts.tile([1, h], F32R, tag="bv")
            bo_sb = consts.tile([1, h], F32R, tag="bo")
            nc.sync.dma_start(out=bq_sb, in_=bq)
            nc.sync.dma_start(out=bk_sb, in_=bk)
            nc.sync.dma_start(out=bv_sb, in_=bv)
            nc.sync.dma_start(out=bo_sb, in_=bo)

            # normalized merged-head context, transposed: [128, MB, sq]
            ctxnT = persist.tile([128, MB, sq], F32R, tag="ctxnT")

            loop_cm = tc.For_i(0, reps, 1) if reps else contextlib.nullcontext()
            with loop_cm:
              for _srep in range(sreps):
                for hf in range(2):
                    hoff = hf * WH

                    # ---- resident weights for this half (1 DMA each;
                    # Wv deferred past the first kT tile so the first kpT
                    # matmul isn't queued behind 4MB of weight DMA) ----
                    Wk_sb = persist.tile([128, KB, WH], F32R, tag="wk")
                    Wv_sb = persist.tile([128, KB, WH], F32R, tag="wv")
                    nc.sync.dma_start(out=Wk_sb,
                                      in_=Wk_p[:, :, hoff:hoff + WH])

                    # ---- kp^T projection: [ho(128 x MH), kt] ----
                    kpT = persist.tile([128, MH, s], F32R, tag="kpT")
                    if ag:
                        agki = dramp.tile([128, MH, SL], F32R, tag="agki")
                        agko = dramp.tile([4, 128, MH, SL], F32R, tag="agko")
                    for n in range(SL // WN):
                        kt_t = stream.tile([128, KB, WN], F32R, tag="st2mb",
                                           name="kt_t")
                        nc.sync.dma_start(
                            out=kt_t, in_=kT_p[:, :, n * WN:(n + 1) * WN])
                        for m in range(MH):
                            ps = ps512.tile([128, WN], F32, tag="ps512")
                            for kb in range(KB):
                                nc.tensor.matmul(
                                    ps, Wk_sb[:, kb, m * 128:(m + 1) * 128],
                                    kt_t[:, kb, :], start=(kb == 0),
                                    stop=False)
                            nc.tensor.matmul(
                                ps,
                                bk_sb[0:1, hoff + m * 128:hoff + (m + 1) * 128],
                                ones[0:1, 0:WN], start=False, stop=True)
                            if ag:
                                kst = wqo.tile([128, WN], F32R, tag="kst")
                                nc.vector.tensor_copy(kst, ps)
                                nc.sync.dma_start(
                                    out=agki[:, m, n * WN:(n + 1) * WN],
                                    in_=kst)
                            else:
                                nc.vector.tensor_copy(
                                    kpT[:, m, n * WN:(n + 1) * WN], ps)

                    # ---- vp projection (token-major) + ones column ----
                    # vp[kt-part, ktb, head_local, 0:64] ; [.., 64] = 1.0
                    nc.sync.dma_start(out=Wv_sb,
                                      in_=Wv_p[:, :, hoff:hoff + WH])
                    vp = persist.tile([128, KTB, HPH, hd + 1], BF16, tag="vp")
                    if ag:
                        KTL = SL // 128
                        vps = persist.tile([128, KTL, HPH, hd + 1], BF16,
                                           tag="vps")
                        agvi = dramp.tile([128, KTL, HPH, hd + 1], BF16,
                                          tag="agvi")
                        agvo = dramp.tile([4, 128, KTL, HPH, hd + 1], BF16,
                                          tag="agvo")
                    else:
                        vps = vp
                    for n in range(SL // WN):
                        vt_t = stream.tile([128, KB, WN], F32R, tag="st2mb",
                                           name="vt_t")
                        nc.sync.dma_start(
                            out=vt_t, in_=vT_p[:, :, n * WN:(n + 1) * WN])
                        for st in range(TPW):
                            t = n * TPW + st
                            nc.vector.tensor_copy(vps[:, t, :, hd:hd + 1],
                                                  ones_rep)
                            ps = ps512.tile([128, WH], F32, tag="ps512")
                            for kb in range(KB):
                                nc.tensor.matmul(
                                    ps, vt_t[:, kb, st * 128:(st + 1) * 128],
                                    Wv_sb[:, kb, :], start=(kb == 0),
                                    stop=False)
                            nc.tensor.matmul(ps, ones[0:1, 0:128],
                                             bv_sb[0:1, hoff:hoff + WH],
                                             start=False, stop=True)
                            nc.vector.tensor_copy(
                                vps[:, t, :, 0:hd],
                                ps.rearrange("p (hh d) -> p hh d", d=hd))
                    if ag:
                        nc.sync.dma_start(out=agvi, in_=vps)
                        nc.gpsimd.collective_compute(
                            "AllGather", mybir.AluOpType.bypass,
                            ins=[agki.opt()], outs=[agko.opt()],
                            replica_groups=[[0, 1, 2, 3], [4, 5, 6, 7]])
                        nc.gpsimd.collective_compute(
                            "AllGather", mybir.AluOpType.bypass,
                            ins=[agvi.opt()], outs=[agvo.opt()],
                            replica_groups=[[0, 1, 2, 3], [4, 5, 6, 7]])

                    # ---- qp^T projection: [ho(128 x MH), qt] ----
                    qT_sb = stream.tile([128, KB, sq], F32R, tag="st2mb",
                                        name="qT_sb")
                    nc.sync.dma_start(out=qT_sb, in_=qT_p)
                    qpT = persist.tile([128, MH, sq], F32R, tag="qpT")
                    wq_t = stream.tile([128, KB, WH], F32R, tag="st2mb",
                                       name="wq_t")
                    nc.sync.dma_start(out=wq_t,
                                      in_=Wq_p[:, :, hoff:hoff + WH])
                    for m in range(MH):
                        ps = ps512.tile([128, sq], F32, tag="ps512")
                        for kb in range(KB):
                            nc.tensor.matmul(
                                ps, wq_t[:, kb, m * 128:(m + 1) * 128],
                                qT_sb[:, kb, :], start=(kb == 0), stop=False)
                        nc.tensor.matmul(
                            ps,
                            bq_sb[0:1, hoff + m * 128:hoff + (m + 1) * 128],
                            ones[0:1, 0:sq], start=False, stop=True)
                        nc.vector.tensor_copy(qpT[:, m, :], ps)

                    if ag:
                        nc.sync.dma_start(
                            out=kpT.rearrange("p m (r t) -> p m r t", r=4),
                            in_=agko.rearrange("r p m t -> p m r t"))
                        nc.sync.dma_start(
                            out=vp.rearrange("p (r k) h c -> p r k h c", r=4),
                            in_=agvo.rearrange("r p k h c -> p r k h c"))

                    if upto < 2:
                        # consume proj outputs so DCE keeps them (timing mode)
                        nc.sync.dma_start(out=outT_p[:, 0, :],
                                          in_=kpT[:, 0, 0:sq].bitcast(F32))
                        nc.sync.dma_start(out=outT_p[:, 1, :],
                                          in_=qpT[:, 0, :].bitcast(F32))
                        nc.gpsimd.dma_start(
                            out=outT_p[:, 2, 0:65],
                            in_=vp[:, hf, 0, :])
                        continue
                    # ---- attention, processed in head pairs ----
                    # scores 2 ktb per 2-bank psum tile -> one exp per
                    # [128, 2*sq]; ctx MMs software-pipelined one chunk
                    # behind so ACT streams without PE ping-pong.
                    CH = min(4 if v2 else 2, KTB)  # ktb per chunk
                    n_ps_bufs = 1 if v2 else 2
                    # Two pairs processed CONCURRENTLY: each pair's
                    # scores->exp->ctx chain has ~1us cross-engine latency
                    # bubbles per chunk; the sibling pair's independent
                    # chunks fill them.  The odd pair's ctx accumulators
                    # live in the ps512 pool (idle during attention), so
                    # PSUM stays exactly at 8 banks.
                    PG = 2 if HPH // 2 >= 2 else 1
                    for prp in range(0, HPH // 2, PG):
                        prs = list(range(prp, min(prp + PG, HPH // 2)))
                        pscs = []
                        for pi in range(len(prs)):
                            pool = pscp if pi == 0 else ps512
                            tag = "psc" if pi == 0 else "ps512"
                            pscs.append([
                                pool.tile([hd + 1, sq], F32, tag=tag,
                                          name=f"psc{pi}_{j}")
                                for j in range(2)])
                        prevs = [None for _ in prs]
                        for cc in range(KTB // CH):
                            for pi, pr in enumerate(prs):
                                m = pr
                                psc = pscs[pi]
                                p1s = [ps1024.tile([128, CH, sq], F32,
                                                   tag="ps1024",
                                                   bufs=n_ps_bufs,
                                                   name=f"p1_{pi}_{j}")
                                       for j in range(2)]
                                for i in range(CH):
                                    ktb = cc * CH + i
                                    for j, roff in enumerate((0, 64)):
                                        nc.tensor.matmul(
                                            p1s[j][:, i, :],
                                            kpT[roff:roff + 64, m,
                                                ktb * 128:(ktb + 1) * 128],
                                            qpT[roff:roff + 64, m, :],
                                            start=True, stop=True)
                                ets = []
                                for j in range(2):
                                    if exp_sbuf and (exp_sbuf == "all"
                                                     or (cc + j) % 2 == 0):
                                        sc = exps.tile([128, CH, sq], F32,
                                                       tag="sc_t", bufs=2,
                                                       name=f"sc_{pi}_{j}")
                                        nc.vector.tensor_copy(sc, p1s[j])
                                    else:
                                        sc = p1s[j]
                                    et = exps.tile([128, CH, sq], BF16,
                                                   tag="exp_t",
                                                   bufs=4 if v2 else 8,
                                                   name=f"et_{pi}_{j}")
                                    nc.scalar.activation(
                                        out=et, in_=sc,
                                        func=mybir.ActivationFunctionType.Exp)
                                    ets.append(et)
                                if prevs[pi] is not None:
                                    pcc, pets = prevs[pi]
                                    for j in range(2):
                                        for i in range(CH):
                                            ktb = pcc * CH + i
                                            nc.tensor.matmul(
                                                psc[j],
                                                vp[:, ktb, 2 * pr + j, :],
                                                pets[j][:, i, :],
                                                start=(ktb == 0),
                                                stop=(ktb == KTB - 1))
                                prevs[pi] = (cc, ets)
                        for pi, pr in enumerate(prs):
                            m = pr
                            psc = pscs[pi]
                            pcc, pets = prevs[pi]
                            for j in range(2):
                                for i in range(CH):
                                    ktb = pcc * CH + i
                                    nc.tensor.matmul(
                                        psc[j], vp[:, ktb, 2 * pr + j, :],
                                        pets[j][:, i, :],
                                        start=(ktb == 0),
                                        stop=(ktb == KTB - 1))
                            # normalize: ctxn = ctx * (1/Z), Z-broadcast on
                            # the otherwise-idle gpsimd engine
                            for j, roff in enumerate((0, 64)):
                                zr = zrp.tile([1, sq], F32, tag="zr", bufs=2)
                                with nc.allow_low_precision(
                                        reason="1/Z of softmax; DVE mul"):
                                    nc.vector.reciprocal(
                                        zr, psc[j][hd:hd + 1, :])
                                zb = zrp.tile([64, sq], F32, tag="zb",
                                              bufs=2)
                                nc.gpsimd.partition_broadcast(zb, zr)
                                nc.vector.tensor_mul(
                                    ctxnT[roff:roff + 64, hf * MH + m, :],
                                    psc[j][0:hd, :], zb)

                if upto < 3:
                    if upto == 2:
                        nc.sync.dma_start(out=outT_p[:, 0, :],
                                          in_=ctxnT[:, 0, :].bitcast(F32))
                    continue_marker = True
                # ---- output projection: outT[o, t] ----
                for ob in range(MB if upto >= 3 else 0):
                    wo_t = wqo.tile([128, KB, 128], F32R, tag="wo_t")
                    nc.sync.dma_start(
                        out=wo_t, in_=Wo_p[:, :, ob * 128:(ob + 1) * 128])
                    po = ps512.tile([128, sq], F32, tag="ps512")
                    for mb in range(MB):
                        nc.tensor.matmul(po, wo_t[:, mb, :], ctxnT[:, mb, :],
                                         start=(mb == 0), stop=False)
                    nc.tensor.matmul(po,
                                     bo_sb[0:1, ob * 128:(ob + 1) * 128],
                                     ones[0:1, 0:sq], start=False, stop=True)
                    ot = wqo.tile([128, sq], F32, tag="ot")
                    nc.vector.tensor_copy(ot, po)
                    nc.sync.dma_start(out=outT_p[:, ob, :], in_=ot)

    nc.compile()
    return nc


def shard_inputs(q, k, v, Wq, bq, Wk, bk, Wv, bv, Wo, bo,
                 s=S, h=H, sq=SQ, n_cores=N_CORES, cpg=CPG, ag=False):
    """Host-side sharding: per-core input dicts (numpy, fp32, contiguous)."""
    scale = np.float32(1.0 / np.sqrt(HD))
    c32 = lambda a: np.ascontiguousarray(a, dtype=np.float32)
    Wq_s, bq_s = c32(Wq) * scale, c32(bq).reshape(1, h) * scale
    Wk_c, bk_c = c32(Wk), c32(bk).reshape(1, h)
    Wv_c, bv_c = c32(Wv), c32(bv).reshape(1, h)
    Wo_c, bo_c = c32(Wo), c32(bo).reshape(1, h)
    kT_b = [c32(k[b].T) for b in range(q.shape[0])]
    vT_b = [c32(v[b].T) for b in range(q.shape[0])]
    in_maps = []
    for c in range(n_cores):
        b, r0 = c // cpg, (c % cpg) * sq
        rr = c % cpg
        ksl = kT_b[b] if not ag else c32(kT_b[b][:, rr * (s // 4):(rr + 1) * (s // 4)])
        vsl = vT_b[b] if not ag else c32(vT_b[b][:, rr * (s // 4):(rr + 1) * (s // 4)])
        in_maps.append({
            "qT": c32(q[b, r0:r0 + sq, :].T),
            "kT": ksl, "vT": vsl,
            "Wq": Wq_s, "bq": bq_s, "Wk": Wk_c, "bk": bk_c,
            "Wv": Wv_c, "bv": bv_c, "Wo": Wo_c, "bo": bo_c,
        })
    return in_maps


_NC_CACHE = {}


AG = False  # set True to use the 4-rank AllGather variant


def get_nc():
    if "nc" not in _NC_CACHE:
        _NC_CACHE["nc"] = build_nc(ag=AG)
    return _NC_CACHE["nc"]


def kernel(q, k, v, Wq, bq, Wk, bk, Wv, bv, Wo, bo):
    q, k, v = np.asarray(q), np.asarray(k), np.asarray(v)
    in_maps = shard_inputs(q, k, v, Wq, bq, Wk, bk, Wv, bv, Wo, bo, ag=AG)
    nc = get_nc()
    res = run_bass_kernel_spmd(nc, in_maps, core_ids=list(range(N_CORES)))
    out = np.empty((B, S, H), dtype=np.float32)
    for c in range(N_CORES):
        b, r0 = c // CPG, (c % CPG) * SQ
        out[b, r0:r0 + SQ, :] = res.results[c]["outT"].T
    return out
